# revision 9
# baseline (speedup 1.0000x reference)
"""AnomalyAttention distributed Bass kernel for 8 TRN2 NeuronCores.

Reference computation (n=4096, d=512):
    qkv = x @ W.T                       # [n, d];  Q = K = V = sigma = qkv
    L   = (Q @ K.T) / sqrt(d)           # [n, n]
    S   = softmax(L, axis=0)            # column softmax
    Z   = S @ V                         # [n, d]
    p[i,j]    = |i - j|
    gaussian  = p + |sigma[:,0]|[None,:] * noise      # noise = fixed jax key(42)
    P   = gaussian / gaussian.sum(-1, keepdims=True)  # row normalized
    returns (Z, P)

Sharding: each core owns a 512-row block i_block = [c*512, (c+1)*512).
Logits are built transposed, L.T[j, i_local] (all j on partitions, local i on
free), so the column softmax reduces along the free axis per partition; the
cross-core part of the reduction is a single 16 KiB AllReduce of per-column
partial sums.  Z.T[:, i_block] = qkv.T @ S.T accumulates over j-chunks, with
the natural-layout qkv lhsT tiles fetched via transpose-DMA from a bf16 DRAM
copy of qkv.T.  P rows are produced in natural layout: |i-j| comes from an
on-device iota, gaussian and its row-sum are fused DVE ops, and the final
1/rowsum scale runs on the scalar engine.
"""

import sys

if "/opt/trn_rl_repo" not in sys.path:
    sys.path.insert(0, "/opt/trn_rl_repo")

from contextlib import ExitStack

import numpy as np

import concourse.bass as bass
import concourse.tile as tile
from concourse import bacc, mybir, bass_utils

N = 4096
D = 512
NC = 8
BLK = N // NC          # 512 rows of S / P per core
P = 128                # partitions
F32 = mybir.dt.float32
F32R = mybir.dt.float32r
BF16 = mybir.dt.bfloat16
INV_SQRT_D = 1.0 / np.sqrt(D)

KC = D // P            # 4 contraction chunks of 128
JC = N // P            # 32 j-chunks of 128
NCH = N // 512         # 8 n-chunks of 512
IC = BLK // P          # 4 local i-chunks of 128
HW_ = 1024             # P-path free-dim tile
NH = N // HW_          # free chunks per i-chunk row

_compiled = None


def _build():
    nc = bacc.Bacc("TRN2", target_bir_lowering=False, debug=False, num_devices=NC)

    # Per-core inputs.  xT/wT hold the same data on every core; xTs/noise/ioff
    # are per-core shards.
    xT = nc.dram_tensor("xT", [D, N], F32R, kind="ExternalInput").ap()      # x.T
    xTs = nc.dram_tensor("xTs", [D, BLK], F32R, kind="ExternalInput").ap()  # x.T[:, i_block]
    wT = nc.dram_tensor("wT", [D, D], F32R, kind="ExternalInput").ap()      # W.T
    noise = nc.dram_tensor("noise", [BLK, N], F32, kind="ExternalInput").ap()
    ioff = nc.dram_tensor("ioff", [P, 1], F32, kind="ExternalInput").ap()   # c*BLK

    out_zt = nc.dram_tensor("zt", [D, BLK], F32, kind="ExternalOutput").ap()  # Z.T block
    out_p = nc.dram_tensor("p", [BLK, N], F32, kind="ExternalOutput").ap()    # P rows

    with tile.TileContext(nc) as tc, ExitStack() as big:
        sb = big.enter_context(tc.tile_pool(name="sb", bufs=1))
        psum = big.enter_context(tc.tile_pool(name="psum", bufs=6, space="PSUM"))
        dram = big.enter_context(tc.tile_pool(name="dram", bufs=1, space="DRAM"))
        stream = big.enter_context(tc.tile_pool(name="stream", bufs=8))

        dram_qkvT = dram.tile([D, N], BF16, name="dram_qkvT")

        # qkvT (bf16) stays resident in SBUF: 4 chunks of [128, N]
        qkvT_sb = [
            sb.tile([P, N], BF16, name=f"qkvT{dc}", tag=f"qkvT{dc}") for dc in range(KC)
        ]
        rhs_i = []

        with tc.tile_pool(name="phA", bufs=1) as phA:
            # ---- load W.T, x.T slice ------------------------------------
            wT_sb, xTs_sb = [], []
            for kc in range(KC):
                w = phA.tile([P, D], F32R, name=f"wT{kc}", tag=f"wT{kc}")
                nc.sync.dma_start(w[:], wT[kc * P:(kc + 1) * P, :])
                wT_sb.append(w)
                s = phA.tile([P, BLK], F32R, name=f"xTs{kc}", tag=f"xTs{kc}")
                nc.sync.dma_start(s[:], xTs[kc * P:(kc + 1) * P, :])
                xTs_sb.append(s)

            # ---- qkvT full (f32r matmul) -> SBUF bf16 + DRAM bf16 -------
            # qkvT[dout, n] = sum_din W.T[din, dout] * x.T[din, n]
            for nch in range(NCH):
                xTn = []
                for kc in range(KC):
                    t = phA.tile([P, 512], F32R, name="xTn", tag=f"xTn{kc}", bufs=2)
                    nc.sync.dma_start(
                        t[:], xT[kc * P:(kc + 1) * P, nch * 512:(nch + 1) * 512]
                    )
                    xTn.append(t)
                for dc in range(KC):
                    ps = psum.tile([P, 512], F32, name="ps", tag="ps")
                    for kc in range(KC):
                        nc.tensor.matmul(
                            ps[:],
                            wT_sb[kc][:, dc * P:(dc + 1) * P],
                            xTn[kc][:],
                            start=(kc == 0),
                            stop=(kc == KC - 1),
                        )
                    dst = qkvT_sb[dc][:, nch * 512:(nch + 1) * 512]
                    nc.scalar.copy(dst, ps[:])
                    nc.sync.dma_start(
                        dram_qkvT[dc * P:(dc + 1) * P, nch * 512:(nch + 1) * 512], dst
                    )

            # ---- rhs_i = qkvT[:, i_block] (bf16, SBUF) ------------------
            for dc in range(KC):
                ps = psum.tile([P, BLK], F32, name="ps", tag="ps")
                for kc in range(KC):
                    nc.tensor.matmul(
                        ps[:],
                        wT_sb[kc][:, dc * P:(dc + 1) * P],
                        xTs_sb[kc][:],
                        start=(kc == 0),
                        stop=(kc == KC - 1),
                    )
                rt = sb.tile([P, BLK], BF16, name=f"rhs_i{dc}", tag=f"rhs_i{dc}")
                nc.scalar.copy(rt[:], ps[:])
                rhs_i.append(rt)

        # ---- logits (transposed) + exp + partial column sums --------
        pd = sb.tile([P, JC], F32, name="pd", tag="pd")
        expT = []
        for jc in range(JC):
            ps = psum.tile([P, BLK], F32, name="ps", tag="ps")
            for dc in range(KC):
                nc.tensor.matmul(
                    ps[:],
                    qkvT_sb[dc][:, jc * P:(jc + 1) * P],
                    rhs_i[dc][:],
                    start=(dc == 0), stop=(dc == KC - 1),
                )
            et = sb.tile([P, BLK], BF16, name=f"expT{jc}", tag=f"expT{jc}")
            nc.scalar.activation(
                et[:], ps[:], mybir.ActivationFunctionType.Exp,
                scale=INV_SQRT_D, accum_out=pd[:, jc:jc + 1],
            )
            expT.append(et)

        # ---- AllReduce the softmax denominators ---------------------
        cc_in = dram.tile([P, JC], F32, name="cc_in")
        cc_out = dram.tile([P, JC], F32, name="cc_out", addr_space="Shared")
        nc.sync.dma_start(cc_in[:], pd[:])
        nc.gpsimd.collective_compute(
            "AllReduce",
            mybir.AluOpType.add,
            replica_groups=[list(range(NC))],
            ins=[cc_in[:]],
            outs=[cc_out[:]],
        )
        pd_full = sb.tile([P, JC], F32, name="pd_full", tag="pd_full")
        nc.sync.dma_start(pd_full[:], cc_out[:])
        rd = sb.tile([P, JC], F32, name="rd", tag="rd")
        nc.vector.reciprocal(rd[:], pd_full[:])

        # ---- normalize S.T in place ---------------------------------
        for jc in range(JC):
            nc.vector.tensor_scalar_mul(expT[jc][:], expT[jc][:], rd[:, jc:jc + 1])

        # ---- Z.T block ----------------------------------------------
        # lhsT = qkv[jc*128:(jc+1)*128, dc*128:(dc+1)*128] fetched by
        # transpose-DMA from the bf16 qkv.T in DRAM.
        for dc in range(KC):
            ps = psum.tile([P, BLK], F32, name="ps", tag="ps")
            for jc in range(JC):
                zl = stream.tile([P, P], BF16, name="z_lhsT", tag="z_lhsT", bufs=8)
                nc.sync.dma_start(
                    zl[:],
                    dram_qkvT[dc * P:(dc + 1) * P, jc * P:(jc + 1) * P],
                    transpose=True,
                )
                nc.tensor.matmul(
                    ps[:], zl[:], expT[jc][:],
                    start=(jc == 0), stop=(jc == JC - 1),
                )
            zt = stream.tile([P, BLK], F32, name="zt_cp", tag="zt_cp", bufs=2)
            nc.scalar.copy(zt[:], ps[:])
            nc.sync.dma_start(out_zt[dc * P:(dc + 1) * P, :], zt[:])

        # ---- |sigma| broadcast tile ---------------------------------
        sig_row = sb.tile([1, N], BF16, name="sig_row", tag="sig_row")
        nc.sync.dma_start(sig_row[:], dram_qkvT[0:1, :])
        nc.scalar.activation(sig_row[:], sig_row[:], mybir.ActivationFunctionType.Abs)
        ones = sb.tile([1, P], BF16, name="ones", tag="ones")
        nc.vector.memset(ones[:], 1.0)
        sigbc = sb.tile([P, N], F32, name="sigbc", tag="sigbc")
        for bc in range(NCH):
            pb = psum.tile([P, 512], F32, name="ps", tag="ps")
            nc.tensor.matmul(
                pb[:], ones[:], sig_row[:, bc * 512:(bc + 1) * 512],
                start=True, stop=True,
            )
            nc.vector.tensor_copy(sigbc[:, bc * 512:(bc + 1) * 512], pb[:])

        # ---- ioff ---------------------------------------------------
        ioff_sb = sb.tile([P, 1], F32, name="ioff_sb", tag="ioff_sb")
        nc.sync.dma_start(ioff_sb[:], ioff[:])

        # ---- P rows -------------------------------------------------
        with tc.tile_pool(name="pstream", bufs=2) as pstream:
            for ic in range(IC):
                rs = sb.tile([P, NH], F32, name=f"rs{ic}", tag=f"rs{ic}")
                gaus = []
                for h in range(NH):
                    j0 = h * HW_
                    nz = pstream.tile([P, HW_], F32, name="nz", tag="nz")
                    nc.sync.dma_start(
                        nz[:], noise[ic * P:(ic + 1) * P, j0:j0 + HW_]
                    )
                    pabs = pstream.tile([P, HW_], F32, name="pabs", tag="pabs")
                    nc.gpsimd.iota(
                        pabs[:], pattern=[[-1, HW_]], base=ic * P - j0,
                        channel_multiplier=1, allow_small_or_imprecise_dtypes=True,
                    )
                    nc.scalar.activation(
                        pabs[:], pabs[:], mybir.ActivationFunctionType.Abs,
                        bias=ioff_sb[:, 0:1],
                    )
                    gau = pstream.tile([P, HW_], F32, name="gau", tag="gau", bufs=NH + 1)
                    nc.gpsimd.tensor_tensor(
                        gau[:], nz[:], sigbc[:, j0:j0 + HW_], mybir.AluOpType.mult
                    )
                    nc.vector.scalar_tensor_tensor(
                        gau[:], gau[:], 0.0, pabs[:],
                        op0=mybir.AluOpType.add, op1=mybir.AluOpType.add,
                        accum_out=rs[:, h:h + 1],
                    )
                    gaus.append(gau)
                rsum = sb.tile([P, 1], F32, name=f"rsum{ic}", tag=f"rsum{ic}")
                nc.vector.tensor_reduce(
                    rsum[:], rs[:], axis=mybir.AxisListType.X, op=mybir.AluOpType.add
                )
                rr = sb.tile([P, 1], F32, name=f"rr{ic}", tag=f"rr{ic}")
                nc.vector.reciprocal(rr[:], rsum[:])
                for h in range(NH):
                    j0 = h * HW_
                    nc.scalar.activation(
                        gaus[h][:], gaus[h][:], mybir.ActivationFunctionType.Copy,
                        scale=rr[:, 0:1],
                    )
                    nc.sync.dma_start(
                        out_p[ic * P:(ic + 1) * P, j0:j0 + HW_], gaus[h][:]
                    )

    nc.compile()
    return nc


def _get_compiled():
    global _compiled
    if _compiled is None:
        _compiled = _build()
    return _compiled


def _make_noise():
    import jax
    import jax.numpy as jnp

    return np.asarray(
        jax.random.normal(jax.random.key(42), (N, N), dtype=jnp.float32)
    )


def make_in_maps(x, W, noise):
    xT = np.ascontiguousarray(x.T)
    wT = np.ascontiguousarray(W.T)
    in_maps = []
    for c in range(NC):
        in_maps.append({
            "xT": xT,
            "xTs": np.ascontiguousarray(xT[:, c * BLK:(c + 1) * BLK]),
            "wT": wT,
            "noise": np.ascontiguousarray(noise[c * BLK:(c + 1) * BLK, :]),
            "ioff": np.full((P, 1), c * BLK, dtype=np.float32),
        })
    return in_maps


def assemble(results):
    Z = np.concatenate([results[c]["zt"].T for c in range(NC)], axis=0)
    Pm = np.concatenate([results[c]["p"] for c in range(NC)], axis=0)
    return Z, Pm


def kernel(x, W):
    x = np.ascontiguousarray(np.asarray(x, dtype=np.float32))
    W = np.ascontiguousarray(np.asarray(W, dtype=np.float32))
    noise = _make_noise()
    nc = _get_compiled()
    in_maps = make_in_maps(x, W, noise)
    res = bass_utils.run_bass_kernel_spmd(
        nc, in_maps, core_ids=list(range(NC)), trace=False
    )
    return assemble(res.results)


# revision 12
# speedup vs baseline: 1.7531x; 1.7531x over previous
"""AnomalyAttention distributed Bass kernel for 8 TRN2 NeuronCores.

Reference computation (n=4096, d=512):
    qkv = x @ W.T                       # [n, d];  Q = K = V = sigma = qkv
    L   = (Q @ K.T) / sqrt(d)           # [n, n]
    S   = softmax(L, axis=0)            # column softmax
    Z   = S @ V                         # [n, d]
    p[i,j]    = |i - j|
    gaussian  = p + |sigma[:,0]|[None,:] * noise      # noise = fixed jax key(42)
    P   = gaussian / gaussian.sum(-1, keepdims=True)  # row normalized
    returns (Z, P)

Sharding: each core owns a 512-row block i_block = [c*512, (c+1)*512).
Logits are built transposed, L.T[j, i_local] (all j on partitions, local i on
free), so the column softmax reduces along the free axis per partition; the
cross-core part of the reduction is a single 16 KiB AllReduce of per-column
partial sums.  The P-path (prior matrix) is emitted before the collective so
its engine work hides the AllReduce latency, and the natural-layout qkv
needed by Z.T = qkv.T @ S.T is recomputed into a blocked bf16 DRAM scratch in
the same shadow.  |i-j| comes from an on-device iota + scalar-engine Abs with
per-partition bias; gaussian and its row-sum are fused DVE ops.
"""

import sys

if "/opt/trn_rl_repo" not in sys.path:
    sys.path.insert(0, "/opt/trn_rl_repo")

from contextlib import ExitStack

import numpy as np

import concourse.bass as bass
import concourse.tile as tile
from concourse import bacc, mybir, bass_utils

N = 4096
D = 512
NC = 8
BLK = N // NC          # 512 rows of S / P per core
P = 128                # partitions
F32 = mybir.dt.float32
F32R = mybir.dt.float32r
BF16 = mybir.dt.bfloat16
INV_SQRT_D = 1.0 / np.sqrt(D)

KC = D // P            # 4 contraction chunks of 128
JC = N // P            # 32 j-chunks of 128
NCH = N // 512         # 8 n-chunks of 512
IC = BLK // P          # 4 local i-chunks of 128
HW_ = 1024             # P-path free-dim tile
NH = N // HW_          # free chunks per i-chunk row

_compiled = None


def _build():
    nc = bacc.Bacc("TRN2", target_bir_lowering=False, debug=False, num_devices=NC)

    # Per-core inputs.  xT/wT hold the same data on every core; xTs/noise/ioff
    # are per-core shards.
    xT = nc.dram_tensor("xT", [D, N], F32R, kind="ExternalInput").ap()      # x.T
    xTs = nc.dram_tensor("xTs", [D, BLK], F32R, kind="ExternalInput").ap()  # x.T[:, i_block]
    wT = nc.dram_tensor("wT", [D, D], F32R, kind="ExternalInput").ap()      # W.T
    noise = nc.dram_tensor("noise", [BLK, N], F32, kind="ExternalInput").ap()
    ioff = nc.dram_tensor("ioff", [P, 1], F32, kind="ExternalInput").ap()   # c*BLK

    out_zt = nc.dram_tensor("zt", [D, BLK], F32, kind="ExternalOutput").ap()  # Z.T block
    out_p = nc.dram_tensor("p", [BLK, N], F32, kind="ExternalOutput").ap()    # P rows

    with tile.TileContext(nc) as tc, ExitStack() as big:
        sb = big.enter_context(tc.tile_pool(name="sb", bufs=1))
        psum = big.enter_context(tc.tile_pool(name="psum", bufs=4, space="PSUM"))
        dram = big.enter_context(tc.tile_pool(name="dram", bufs=1, space="DRAM"))

        # blocked natural-layout qkv scratch: block jc = qkv[jc*128:(jc+1)*128, :]
        dram_qkv = dram.tile([N, D], BF16, name="dram_qkv")

        # qkvT (bf16) stays resident in SBUF: 4 chunks of [128, N]
        qkvT_sb = [
            sb.tile([P, N], BF16, name=f"qkvT{dc}", tag=f"qkvT{dc}") for dc in range(KC)
        ]
        rhs_i = []
        pd = sb.tile([P, JC], F32, name="pd", tag="pd")
        expT = []

        with tc.tile_pool(name="phA", bufs=1) as phA:
            # ---- load W.T, x.T slice ------------------------------------
            wT_sb, xTs_sb = [], []
            for kc in range(KC):
                w = phA.tile([P, D], F32R, name=f"wT{kc}", tag=f"wT{kc}")
                nc.sync.dma_start(w[:], wT[kc * P:(kc + 1) * P, :])
                wT_sb.append(w)
                s = phA.tile([P, BLK], F32R, name=f"xTs{kc}", tag=f"xTs{kc}")
                nc.sync.dma_start(s[:], xTs[kc * P:(kc + 1) * P, :])
                xTs_sb.append(s)

            # ---- rhs_i = qkvT[:, i_block] (bf16, SBUF) ------------------
            for dc in range(KC):
                ps = psum.tile([P, BLK], F32, name="ps", tag="ps")
                for kc in range(KC):
                    nc.tensor.matmul(
                        ps[:],
                        wT_sb[kc][:, dc * P:(dc + 1) * P],
                        xTs_sb[kc][:],
                        start=(kc == 0),
                        stop=(kc == KC - 1),
                    )
                rt = sb.tile([P, BLK], BF16, name=f"rhs_i{dc}", tag=f"rhs_i{dc}")
                nc.scalar.copy(rt[:], ps[:])
                rhs_i.append(rt)

            # ---- qkvT full (f32r matmul) -> SBUF bf16, interleaved with
            # ---- logits + exp + partial column sums ---------------------
            xTn_tiles = {}
            for nch in range(NCH):
                xTn = []
                for kc in range(KC):
                    t = phA.tile([P, 512], F32R, name="xTn", tag=f"xTn{kc}", bufs=NCH)
                    nc.sync.dma_start(
                        t[:], xT[kc * P:(kc + 1) * P, nch * 512:(nch + 1) * 512]
                    )
                    xTn.append(t)
                xTn_tiles[nch] = xTn
                for dc in range(KC):
                    ps = psum.tile([P, 512], F32, name="ps", tag="ps")
                    for kc in range(KC):
                        nc.tensor.matmul(
                            ps[:],
                            wT_sb[kc][:, dc * P:(dc + 1) * P],
                            xTn[kc][:],
                            start=(kc == 0),
                            stop=(kc == KC - 1),
                        )
                    nc.scalar.copy(qkvT_sb[dc][:, nch * 512:(nch + 1) * 512], ps[:])
                # logits for the 4 j-chunks covered by this n-chunk
                for jl in range(4):
                    jc = nch * 4 + jl
                    ps = psum.tile([P, BLK], F32, name="ps", tag="ps")
                    for dc in range(KC):
                        nc.tensor.matmul(
                            ps[:],
                            qkvT_sb[dc][:, jc * P:(jc + 1) * P],
                            rhs_i[dc][:],
                            start=(dc == 0), stop=(dc == KC - 1),
                        )
                    et = sb.tile([P, BLK], BF16, name=f"expT{jc}", tag=f"expT{jc}")
                    nc.scalar.activation(
                        et[:], ps[:], mybir.ActivationFunctionType.Exp,
                        scale=INV_SQRT_D, accum_out=pd[:, jc:jc + 1],
                    )
                    expT.append(et)

            # ---- qkv natural (f32r matmul) -> blocked bf16 DRAM ---------
            # qkv[n, dout] = sum_din x.T[din, n] * W.T[din, dout]
            # Emitted after logits so it runs in the AllReduce shadow.
            with tc.tile_pool(name="qnat", bufs=3) as qnat:
                for nch in range(NCH):
                    for jl in range(4):
                        jc = nch * 4 + jl
                        ps = psum.tile([P, D], F32, name="ps", tag="ps")
                        for kc in range(KC):
                            nc.tensor.matmul(
                                ps[:],
                                xTn_tiles[nch][kc][:, jl * P:(jl + 1) * P],
                                wT_sb[kc][:],
                                start=(kc == 0),
                                stop=(kc == KC - 1),
                            )
                        qn = qnat.tile([P, D], BF16, name="qn_cp", tag="qn_cp")
                        nc.vector.tensor_copy(qn[:], ps[:])
                        nc.sync.dma_start(dram_qkv[jc * P:(jc + 1) * P, :], qn[:])

        # ---- |sigma| broadcast tile ---------------------------------
        sig_row = sb.tile([1, N], BF16, name="sig_row", tag="sig_row")
        nc.scalar.activation(
            sig_row[:], qkvT_sb[0][0:1, :], mybir.ActivationFunctionType.Abs
        )
        ones = sb.tile([1, P], BF16, name="ones", tag="ones")
        nc.vector.memset(ones[:], 1.0)
        sigbc = sb.tile([P, N], F32, name="sigbc", tag="sigbc")
        for bc in range(NCH):
            pb = psum.tile([P, 512], F32, name="ps", tag="ps")
            nc.tensor.matmul(
                pb[:], ones[:], sig_row[:, bc * 512:(bc + 1) * 512],
                start=True, stop=True,
            )
            nc.vector.tensor_copy(sigbc[:, bc * 512:(bc + 1) * 512], pb[:])

        # ---- ioff ---------------------------------------------------
        ioff_sb = sb.tile([P, 1], F32, name="ioff_sb", tag="ioff_sb")
        nc.sync.dma_start(ioff_sb[:], ioff[:])

        # ---- P rows (emitted before the AllReduce: hides its latency)
        with tc.tile_pool(name="pstream", bufs=2) as pstream:
            for ic in range(IC):
                rs = sb.tile([P, NH], F32, name=f"rs{ic}", tag=f"rs{ic}")
                gaus = []
                for h in range(NH):
                    j0 = h * HW_
                    nz = pstream.tile([P, HW_], F32, name="nz", tag="nz")
                    nc.sync.dma_start(
                        nz[:], noise[ic * P:(ic + 1) * P, j0:j0 + HW_]
                    )
                    pabs = pstream.tile([P, HW_], F32, name="pabs", tag="pabs")
                    nc.gpsimd.iota(
                        pabs[:], pattern=[[-1, HW_]], base=ic * P - j0,
                        channel_multiplier=1, allow_small_or_imprecise_dtypes=True,
                    )
                    nc.scalar.activation(
                        pabs[:], pabs[:], mybir.ActivationFunctionType.Abs,
                        bias=ioff_sb[:, 0:1],
                    )
                    gau = pstream.tile([P, HW_], F32, name="gau", tag="gau", bufs=NH + 1)
                    nc.gpsimd.tensor_tensor(
                        gau[:], nz[:], sigbc[:, j0:j0 + HW_], mybir.AluOpType.mult
                    )
                    nc.vector.scalar_tensor_tensor(
                        gau[:], gau[:], 0.0, pabs[:],
                        op0=mybir.AluOpType.add, op1=mybir.AluOpType.add,
                        accum_out=rs[:, h:h + 1],
                    )
                    gaus.append(gau)
                rsum = sb.tile([P, 1], F32, name=f"rsum{ic}", tag=f"rsum{ic}")
                nc.vector.tensor_reduce(
                    rsum[:], rs[:], axis=mybir.AxisListType.X, op=mybir.AluOpType.add
                )
                rr = sb.tile([P, 1], F32, name=f"rr{ic}", tag=f"rr{ic}")
                nc.vector.reciprocal(rr[:], rsum[:])
                for h in range(NH):
                    j0 = h * HW_
                    nc.scalar.activation(
                        gaus[h][:], gaus[h][:], mybir.ActivationFunctionType.Copy,
                        scale=rr[:, 0:1],
                    )
                    nc.sync.dma_start(
                        out_p[ic * P:(ic + 1) * P, j0:j0 + HW_], gaus[h][:]
                    )

        # ---- AllReduce the softmax denominators ---------------------
        cc_in = dram.tile([P, JC], F32, name="cc_in")
        cc_out = dram.tile([P, JC], F32, name="cc_out", addr_space="Shared")
        nc.sync.dma_start(cc_in[:], pd[:])
        nc.gpsimd.collective_compute(
            "AllReduce",
            mybir.AluOpType.add,
            replica_groups=[list(range(NC))],
            ins=[cc_in[:]],
            outs=[cc_out[:]],
        )
        pd_full = sb.tile([P, JC], F32, name="pd_full", tag="pd_full")
        nc.sync.dma_start(pd_full[:], cc_out[:])
        rd = sb.tile([P, JC], F32, name="rd", tag="rd")
        nc.vector.reciprocal(rd[:], pd_full[:])

        # ---- normalize S.T in place ---------------------------------
        for jc in range(JC):
            nc.vector.tensor_scalar_mul(expT[jc][:], expT[jc][:], rd[:, jc:jc + 1])

        # ---- Z.T block: 4 PSUM banks accumulate in parallel ---------
        psz = [
            psum.tile([P, BLK], F32, name=f"psz{dc}", tag=f"psz{dc}", bufs=1)
            for dc in range(KC)
        ]
        with tc.tile_pool(name="zstream", bufs=4) as zstream:
            for jc in range(JC):
                qn = zstream.tile([P, D], BF16, name="z_qn", tag="z_qn")
                nc.sync.dma_start(qn[:], dram_qkv[jc * P:(jc + 1) * P, :])
                for dc in range(KC):
                    nc.tensor.matmul(
                        psz[dc][:], qn[:, dc * P:(dc + 1) * P], expT[jc][:],
                        start=(jc == 0), stop=(jc == JC - 1),
                    )
            for dc in range(KC):
                zt = zstream.tile([P, BLK], F32, name="zt_cp", tag="zt_cp", bufs=2)
                nc.scalar.copy(zt[:], psz[dc][:])
                nc.sync.dma_start(out_zt[dc * P:(dc + 1) * P, :], zt[:])

    nc.compile()
    return nc


def _get_compiled():
    global _compiled
    if _compiled is None:
        _compiled = _build()
    return _compiled


def _make_noise():
    import jax
    import jax.numpy as jnp

    return np.asarray(
        jax.random.normal(jax.random.key(42), (N, N), dtype=jnp.float32)
    )


def make_in_maps(x, W, noise):
    xT = np.ascontiguousarray(x.T)
    wT = np.ascontiguousarray(W.T)
    in_maps = []
    for c in range(NC):
        in_maps.append({
            "xT": xT,
            "xTs": np.ascontiguousarray(xT[:, c * BLK:(c + 1) * BLK]),
            "wT": wT,
            "noise": np.ascontiguousarray(noise[c * BLK:(c + 1) * BLK, :]),
            "ioff": np.full((P, 1), c * BLK, dtype=np.float32),
        })
    return in_maps


def assemble(results):
    Z = np.concatenate([results[c]["zt"].T for c in range(NC)], axis=0)
    Pm = np.concatenate([results[c]["p"] for c in range(NC)], axis=0)
    return Z, Pm


def kernel(x, W):
    x = np.ascontiguousarray(np.asarray(x, dtype=np.float32))
    W = np.ascontiguousarray(np.asarray(W, dtype=np.float32))
    noise = _make_noise()
    nc = _get_compiled()
    in_maps = make_in_maps(x, W, noise)
    res = bass_utils.run_bass_kernel_spmd(
        nc, in_maps, core_ids=list(range(NC)), trace=False
    )
    return assemble(res.results)


# revision 13
# speedup vs baseline: 1.8459x; 1.0530x over previous
"""AnomalyAttention distributed Bass kernel for 8 TRN2 NeuronCores.

Reference computation (n=4096, d=512):
    qkv = x @ W.T                       # [n, d];  Q = K = V = sigma = qkv
    L   = (Q @ K.T) / sqrt(d)           # [n, n]
    S   = softmax(L, axis=0)            # column softmax
    Z   = S @ V                         # [n, d]
    p[i,j]    = |i - j|
    gaussian  = p + |sigma[:,0]|[None,:] * noise      # noise = fixed jax key(42)
    P   = gaussian / gaussian.sum(-1, keepdims=True)  # row normalized
    returns (Z, P)

Sharding: each core owns a 512-row block i_block = [c*512, (c+1)*512).
Logits are built transposed, L.T[j, i_local] (all j on partitions, local i on
free), so the column softmax reduces along the free axis per partition; the
cross-core part of the reduction is a single 16 KiB AllReduce of per-column
partial sums, hidden behind the P-path (prior matrix) work.  Both layouts of
qkv (transposed for the logits lhsT, natural for Z.T = qkv.T @ S.T) are
computed on-chip in bf16 and stay SBUF-resident.  |i-j| comes from an
on-device iota + scalar-engine Abs with per-partition bias; gaussian and its
row-sum are fused DVE ops; the 1/rowsum scale runs on the scalar engine.
"""

import sys

if "/opt/trn_rl_repo" not in sys.path:
    sys.path.insert(0, "/opt/trn_rl_repo")

from contextlib import ExitStack

import ml_dtypes
import numpy as np

import concourse.bass as bass
import concourse.tile as tile
from concourse import bacc, mybir, bass_utils

N = 4096
D = 512
NC = 8
BLK = N // NC          # 512 rows of S / P per core
P = 128                # partitions
F32 = mybir.dt.float32
BF16 = mybir.dt.bfloat16
INV_SQRT_D = 1.0 / np.sqrt(D)

KC = D // P            # 4 contraction chunks of 128
JC = N // P            # 32 j-chunks of 128
NCH = N // 512         # 8 n-chunks of 512
IC = BLK // P          # 4 local i-chunks of 128
HW_ = 1024             # P-path free-dim tile
NH = N // HW_          # free chunks per i-chunk row

_compiled = None


def _build():
    nc = bacc.Bacc("TRN2", target_bir_lowering=False, debug=False, num_devices=NC)

    # Per-core inputs (bf16 except the P-path data).  xT/wT hold the same
    # data on every core; xTs/noise/ioff are per-core shards.
    xT = nc.dram_tensor("xT", [D, N], BF16, kind="ExternalInput").ap()      # x.T
    xTs = nc.dram_tensor("xTs", [D, BLK], BF16, kind="ExternalInput").ap()  # x.T[:, i_block]
    wT = nc.dram_tensor("wT", [D, D], BF16, kind="ExternalInput").ap()      # W.T
    noise = nc.dram_tensor("noise", [BLK, N], F32, kind="ExternalInput").ap()
    ioff = nc.dram_tensor("ioff", [P, 1], F32, kind="ExternalInput").ap()   # c*BLK

    out_zt = nc.dram_tensor("zt", [D, BLK], F32, kind="ExternalOutput").ap()  # Z.T block
    out_p = nc.dram_tensor("p", [BLK, N], F32, kind="ExternalOutput").ap()    # P rows

    with tile.TileContext(nc) as tc, ExitStack() as big:
        sb = big.enter_context(tc.tile_pool(name="sb", bufs=1))
        psum = big.enter_context(tc.tile_pool(name="psum", bufs=4, space="PSUM"))
        dram = big.enter_context(tc.tile_pool(name="dram", bufs=1, space="DRAM"))

        # bf16 SBUF residents: qkvT (4 x [128, N]) and natural qkv (32 x [128, D])
        qkvT_sb = [
            sb.tile([P, N], BF16, name=f"qkvT{dc}", tag=f"qkvT{dc}") for dc in range(KC)
        ]
        qn_sb = [
            sb.tile([P, D], BF16, name=f"qn{jc}", tag=f"qn{jc}") for jc in range(JC)
        ]
        sigbc = sb.tile([P, N], BF16, name="sigbc", tag="sigbc")
        ones = sb.tile([1, P], BF16, name="ones", tag="ones")
        nc.vector.memset(ones[:], 1.0)
        ioff_sb = sb.tile([P, 1], F32, name="ioff_sb", tag="ioff_sb")
        nc.sync.dma_start(ioff_sb[:], ioff[:])
        pd = sb.tile([P, JC], F32, name="pd", tag="pd")
        rhs_i = []
        expT = []

        with tc.tile_pool(name="phA", bufs=1) as phA:
            # ---- load W.T, x.T slice ------------------------------------
            wT_sb, xTs_sb = [], []
            for kc in range(KC):
                w = phA.tile([P, D], BF16, name=f"wT{kc}", tag=f"wT{kc}")
                nc.sync.dma_start(w[:], wT[kc * P:(kc + 1) * P, :])
                wT_sb.append(w)
                s = phA.tile([P, BLK], BF16, name=f"xTs{kc}", tag=f"xTs{kc}")
                nc.sync.dma_start(s[:], xTs[kc * P:(kc + 1) * P, :])
                xTs_sb.append(s)

            # ---- rhs_i = qkvT[:, i_block] (bf16, SBUF) ------------------
            for dc in range(KC):
                ps = psum.tile([P, BLK], F32, name="ps", tag="ps")
                for kc in range(KC):
                    nc.tensor.matmul(
                        ps[:],
                        wT_sb[kc][:, dc * P:(dc + 1) * P],
                        xTs_sb[kc][:],
                        start=(kc == 0),
                        stop=(kc == KC - 1),
                    )
                rt = sb.tile([P, BLK], BF16, name=f"rhs_i{dc}", tag=f"rhs_i{dc}")
                nc.scalar.copy(rt[:], ps[:])
                rhs_i.append(rt)

            # ---- qkvT full -> SBUF bf16, interleaved with logits + exp +
            # ---- partial column sums + per-chunk |sigma| broadcast ------
            xTn_tiles = {}
            for nch in range(NCH):
                xTn = []
                for kc in range(KC):
                    t = phA.tile([P, 512], BF16, name="xTn", tag=f"xTn{kc}", bufs=NCH)
                    nc.sync.dma_start(
                        t[:], xT[kc * P:(kc + 1) * P, nch * 512:(nch + 1) * 512]
                    )
                    xTn.append(t)
                xTn_tiles[nch] = xTn
                for dc in range(KC):
                    ps = psum.tile([P, 512], F32, name="ps", tag="ps")
                    for kc in range(KC):
                        nc.tensor.matmul(
                            ps[:],
                            wT_sb[kc][:, dc * P:(dc + 1) * P],
                            xTn[kc][:],
                            start=(kc == 0),
                            stop=(kc == KC - 1),
                        )
                    nc.scalar.copy(qkvT_sb[dc][:, nch * 512:(nch + 1) * 512], ps[:])
                # |sigma| broadcast chunk: row 0 of qkvT -> all 128 partitions
                sg = phA.tile([1, 512], BF16, name="sg", tag="sg", bufs=2)
                nc.scalar.activation(
                    sg[:], qkvT_sb[0][0:1, nch * 512:(nch + 1) * 512],
                    mybir.ActivationFunctionType.Abs,
                )
                pb = psum.tile([P, 512], F32, name="ps", tag="ps")
                nc.tensor.matmul(pb[:], ones[:], sg[:], start=True, stop=True)
                nc.vector.tensor_copy(sigbc[:, nch * 512:(nch + 1) * 512], pb[:])
                # logits for the 4 j-chunks covered by this n-chunk
                for jl in range(4):
                    jc = nch * 4 + jl
                    ps = psum.tile([P, BLK], F32, name="ps", tag="ps")
                    for dc in range(KC):
                        nc.tensor.matmul(
                            ps[:],
                            qkvT_sb[dc][:, jc * P:(jc + 1) * P],
                            rhs_i[dc][:],
                            start=(dc == 0), stop=(dc == KC - 1),
                        )
                    et = sb.tile([P, BLK], BF16, name=f"expT{jc}", tag=f"expT{jc}")
                    nc.scalar.activation(
                        et[:], ps[:], mybir.ActivationFunctionType.Exp,
                        scale=INV_SQRT_D, accum_out=pd[:, jc:jc + 1],
                    )
                    expT.append(et)

            # ---- natural qkv -> SBUF bf16 (runs in the AllReduce shadow)
            # qkv[n, dout] = sum_din x.T[din, n] * W.T[din, dout]
            for nch in range(NCH):
                for jl in range(4):
                    jc = nch * 4 + jl
                    ps = psum.tile([P, D], F32, name="ps", tag="ps")
                    for kc in range(KC):
                        nc.tensor.matmul(
                            ps[:],
                            xTn_tiles[nch][kc][:, jl * P:(jl + 1) * P],
                            wT_sb[kc][:],
                            start=(kc == 0),
                            stop=(kc == KC - 1),
                        )
                    nc.any.tensor_copy(qn_sb[jc][:], ps[:])

        # ---- P rows (emitted before the AllReduce: hides its latency)
        with tc.tile_pool(name="pstream", bufs=2) as pstream:
            for ic in range(IC):
                rs = sb.tile([P, NH], F32, name=f"rs{ic}", tag=f"rs{ic}")
                gaus = []
                for h in range(NH):
                    j0 = h * HW_
                    nz = pstream.tile([P, HW_], F32, name="nz", tag="nz")
                    nc.sync.dma_start(
                        nz[:], noise[ic * P:(ic + 1) * P, j0:j0 + HW_]
                    )
                    pabs = pstream.tile([P, HW_], F32, name="pabs", tag="pabs")
                    nc.gpsimd.iota(
                        pabs[:], pattern=[[-1, HW_]], base=ic * P - j0,
                        channel_multiplier=1, allow_small_or_imprecise_dtypes=True,
                    )
                    nc.scalar.activation(
                        pabs[:], pabs[:], mybir.ActivationFunctionType.Abs,
                        bias=ioff_sb[:, 0:1],
                    )
                    gau = pstream.tile([P, HW_], F32, name="gau", tag="gau", bufs=NH + 1)
                    nc.gpsimd.tensor_tensor(
                        gau[:], nz[:], sigbc[:, j0:j0 + HW_], mybir.AluOpType.mult
                    )
                    nc.vector.scalar_tensor_tensor(
                        gau[:], gau[:], 0.0, pabs[:],
                        op0=mybir.AluOpType.add, op1=mybir.AluOpType.add,
                        accum_out=rs[:, h:h + 1],
                    )
                    gaus.append(gau)
                rsum = sb.tile([P, 1], F32, name=f"rsum{ic}", tag=f"rsum{ic}")
                nc.vector.tensor_reduce(
                    rsum[:], rs[:], axis=mybir.AxisListType.X, op=mybir.AluOpType.add
                )
                rr = sb.tile([P, 1], F32, name=f"rr{ic}", tag=f"rr{ic}")
                nc.vector.reciprocal(rr[:], rsum[:])
                for h in range(NH):
                    j0 = h * HW_
                    nc.scalar.activation(
                        gaus[h][:], gaus[h][:], mybir.ActivationFunctionType.Copy,
                        scale=rr[:, 0:1],
                    )
                    nc.sync.dma_start(
                        out_p[ic * P:(ic + 1) * P, j0:j0 + HW_], gaus[h][:]
                    )

        # ---- AllReduce the softmax denominators ---------------------
        cc_in = dram.tile([P, JC], F32, name="cc_in")
        cc_out = dram.tile([P, JC], F32, name="cc_out", addr_space="Shared")
        nc.sync.dma_start(cc_in[:], pd[:])
        nc.gpsimd.collective_compute(
            "AllReduce",
            mybir.AluOpType.add,
            replica_groups=[list(range(NC))],
            ins=[cc_in[:]],
            outs=[cc_out[:]],
        )
        pd_full = sb.tile([P, JC], F32, name="pd_full", tag="pd_full")
        nc.sync.dma_start(pd_full[:], cc_out[:])
        rd = sb.tile([P, JC], F32, name="rd", tag="rd")
        nc.vector.reciprocal(rd[:], pd_full[:])

        # ---- normalize S.T in place ---------------------------------
        for jc in range(JC):
            nc.vector.tensor_scalar_mul(expT[jc][:], expT[jc][:], rd[:, jc:jc + 1])

        # ---- Z.T block: 4 PSUM banks accumulate in parallel ---------
        psz = [
            psum.tile([P, BLK], F32, name=f"psz{dc}", tag=f"psz{dc}", bufs=1)
            for dc in range(KC)
        ]
        for jc in range(JC):
            for dc in range(KC):
                nc.tensor.matmul(
                    psz[dc][:], qn_sb[jc][:, dc * P:(dc + 1) * P], expT[jc][:],
                    start=(jc == 0), stop=(jc == JC - 1),
                )
        with tc.tile_pool(name="zout", bufs=2) as zout:
            for dc in range(KC):
                zt = zout.tile([P, BLK], F32, name="zt_cp", tag="zt_cp")
                nc.scalar.copy(zt[:], psz[dc][:])
                nc.sync.dma_start(out_zt[dc * P:(dc + 1) * P, :], zt[:])

    nc.compile()
    return nc


def _get_compiled():
    global _compiled
    if _compiled is None:
        _compiled = _build()
    return _compiled


def _make_noise():
    import jax
    import jax.numpy as jnp

    return np.asarray(
        jax.random.normal(jax.random.key(42), (N, N), dtype=jnp.float32)
    )


def make_in_maps(x, W, noise):
    bf = ml_dtypes.bfloat16
    xT = np.ascontiguousarray(x.T.astype(bf))
    wT = np.ascontiguousarray(W.T.astype(bf))
    in_maps = []
    for c in range(NC):
        in_maps.append({
            "xT": xT,
            "xTs": np.ascontiguousarray(xT[:, c * BLK:(c + 1) * BLK]),
            "wT": wT,
            "noise": np.ascontiguousarray(noise[c * BLK:(c + 1) * BLK, :]),
            "ioff": np.full((P, 1), c * BLK, dtype=np.float32),
        })
    return in_maps


def assemble(results):
    Z = np.concatenate([results[c]["zt"].T for c in range(NC)], axis=0)
    Pm = np.concatenate([results[c]["p"] for c in range(NC)], axis=0)
    return Z, Pm


def kernel(x, W):
    x = np.ascontiguousarray(np.asarray(x, dtype=np.float32))
    W = np.ascontiguousarray(np.asarray(W, dtype=np.float32))
    noise = _make_noise()
    nc = _get_compiled()
    in_maps = make_in_maps(x, W, noise)
    res = bass_utils.run_bass_kernel_spmd(
        nc, in_maps, core_ids=list(range(NC)), trace=False
    )
    return assemble(res.results)


# revision 14
# speedup vs baseline: 2.1040x; 1.1398x over previous
"""AnomalyAttention distributed Bass kernel for 8 TRN2 NeuronCores.

Reference computation (n=4096, d=512):
    qkv = x @ W.T                       # [n, d];  Q = K = V = sigma = qkv
    L   = (Q @ K.T) / sqrt(d)           # [n, n]
    S   = softmax(L, axis=0)            # column softmax
    Z   = S @ V                         # [n, d]
    p[i,j]    = |i - j|
    gaussian  = p + |sigma[:,0]|[None,:] * noise      # noise = fixed jax key(42)
    P   = gaussian / gaussian.sum(-1, keepdims=True)  # row normalized
    returns (Z, P)

Sharding: each core owns a 512-row block i_block = [c*512, (c+1)*512).
Logits are built transposed, L.T[j, i_local] (all j on partitions, local i on
free), so the column softmax reduces along the free axis per partition; the
cross-core part of the reduction is a single 16 KiB AllReduce of per-column
partial sums, hidden behind the P-path (prior matrix) work.  Both layouts of
qkv (transposed for the logits lhsT, natural for Z.T = qkv.T @ S.T) are
computed on-chip in bf16 and stay SBUF-resident.  |i-j| comes from an
on-device iota + scalar-engine Abs with per-partition bias; gaussian and its
row-sum are fused DVE ops; the 1/rowsum scale runs on the scalar engine.
"""

import sys

if "/opt/trn_rl_repo" not in sys.path:
    sys.path.insert(0, "/opt/trn_rl_repo")

from contextlib import ExitStack

import ml_dtypes
import numpy as np

import concourse.bass as bass
import concourse.tile as tile
from concourse import bacc, mybir, bass_utils

N = 4096
D = 512
NC = 8
BLK = N // NC          # 512 rows of S / P per core
P = 128                # partitions
F32 = mybir.dt.float32
BF16 = mybir.dt.bfloat16
INV_SQRT_D = 1.0 / np.sqrt(D)

KC = D // P            # 4 contraction chunks of 128
JC = N // P            # 32 j-chunks of 128
NCH = N // 512         # 8 n-chunks of 512
IC = BLK // P          # 4 local i-chunks of 128
HW_ = 1024             # P-path free-dim tile
NH = N // HW_          # free chunks per i-chunk row

_compiled = None


def _build():
    nc = bacc.Bacc("TRN2", target_bir_lowering=False, debug=False, num_devices=NC)

    # Per-core inputs (bf16 except the P-path data).  xT/wT hold the same
    # data on every core; xTs/noise/ioff are per-core shards.
    xT = nc.dram_tensor("xT", [D, N], BF16, kind="ExternalInput").ap()      # x.T
    xTs = nc.dram_tensor("xTs", [D, BLK], BF16, kind="ExternalInput").ap()  # x.T[:, i_block]
    wT = nc.dram_tensor("wT", [D, D], BF16, kind="ExternalInput").ap()      # W.T
    noise = nc.dram_tensor("noise", [BLK, N], F32, kind="ExternalInput").ap()
    ioff = nc.dram_tensor("ioff", [P, 1], F32, kind="ExternalInput").ap()   # c*BLK

    out_zt = nc.dram_tensor("zt", [D, BLK], F32, kind="ExternalOutput").ap()  # Z.T block
    out_p = nc.dram_tensor("p", [BLK, N], F32, kind="ExternalOutput").ap()    # P rows

    with tile.TileContext(nc) as tc, ExitStack() as big:
        sb = big.enter_context(tc.tile_pool(name="sb", bufs=1))
        psum = big.enter_context(tc.tile_pool(name="psum", bufs=4, space="PSUM"))
        dram = big.enter_context(tc.tile_pool(name="dram", bufs=1, space="DRAM"))
        pstream = big.enter_context(tc.tile_pool(name="pstream", bufs=2))

        # bf16 SBUF residents: qkvT (4 x [128, N]) and natural qkv (32 x [128, D])
        qkvT_sb = [
            sb.tile([P, N], BF16, name=f"qkvT{dc}", tag=f"qkvT{dc}") for dc in range(KC)
        ]
        qn_sb = [
            sb.tile([P, D], BF16, name=f"qn{jc}", tag=f"qn{jc}") for jc in range(JC)
        ]
        sigbc = sb.tile([P, N], BF16, name="sigbc", tag="sigbc")
        ones = sb.tile([1, P], BF16, name="ones", tag="ones")
        nc.vector.memset(ones[:], 1.0)
        ioff_sb = sb.tile([P, 1], F32, name="ioff_sb", tag="ioff_sb")
        nc.sync.dma_start(ioff_sb[:], ioff[:])
        pd = sb.tile([P, JC], F32, name="pd", tag="pd")
        rhs_i = []
        expT = []

        with tc.tile_pool(name="phA", bufs=1) as phA:
            # ---- load W.T, x.T slice ------------------------------------
            wT_sb, xTs_sb = [], []
            for kc in range(KC):
                w = phA.tile([P, D], BF16, name=f"wT{kc}", tag=f"wT{kc}")
                nc.sync.dma_start(w[:], wT[kc * P:(kc + 1) * P, :])
                wT_sb.append(w)
                s = phA.tile([P, BLK], BF16, name=f"xTs{kc}", tag=f"xTs{kc}")
                nc.sync.dma_start(s[:], xTs[kc * P:(kc + 1) * P, :])
                xTs_sb.append(s)

            # ---- rhs_i = qkvT[:, i_block] (bf16, SBUF) ------------------
            for dc in range(KC):
                ps = psum.tile([P, BLK], F32, name="ps", tag="ps")
                for kc in range(KC):
                    nc.tensor.matmul(
                        ps[:],
                        wT_sb[kc][:, dc * P:(dc + 1) * P],
                        xTs_sb[kc][:],
                        start=(kc == 0),
                        stop=(kc == KC - 1),
                    )
                rt = sb.tile([P, BLK], BF16, name=f"rhs_i{dc}", tag=f"rhs_i{dc}")
                nc.scalar.copy(rt[:], ps[:])
                rhs_i.append(rt)

            # ---- natural qkv shard -> AllGather -> qn_sb -----------------
            # qkv[i_block, :] = x[i_block, :] @ W.T, gathered across cores.
            ag_in = dram.tile([BLK, D], BF16, name="ag_in")
            ag_out = dram.tile([N, D], BF16, name="ag_out", addr_space="Shared")
            for jl in range(4):
                ps = psum.tile([P, D], F32, name="ps", tag="ps")
                for kc in range(KC):
                    nc.tensor.matmul(
                        ps[:],
                        xTs_sb[kc][:, jl * P:(jl + 1) * P],
                        wT_sb[kc][:],
                        start=(kc == 0),
                        stop=(kc == KC - 1),
                    )
                qs = phA.tile([P, D], BF16, name="qshard", tag="qshard", bufs=2)
                nc.scalar.copy(qs[:], ps[:])
                nc.sync.dma_start(ag_in[jl * P:(jl + 1) * P, :], qs[:])
            nc.gpsimd.collective_compute(
                "AllGather",
                mybir.AluOpType.bypass,
                replica_groups=[list(range(NC))],
                ins=[ag_in[:]],
                outs=[ag_out[:]],
            )
            for jc in range(JC):
                nc.sync.dma_start(qn_sb[jc][:], ag_out[jc * P:(jc + 1) * P, :])

            # ---- qkvT full -> SBUF bf16, interleaved with logits + exp +
            # ---- partial column sums + per-chunk |sigma| broadcast ------
            for nch in range(NCH):
                xTn = []
                for kc in range(KC):
                    t = phA.tile([P, 512], BF16, name="xTn", tag=f"xTn{kc}", bufs=3)
                    nc.sync.dma_start(
                        t[:], xT[kc * P:(kc + 1) * P, nch * 512:(nch + 1) * 512]
                    )
                    xTn.append(t)
                for dc in range(KC):
                    ps = psum.tile([P, 512], F32, name="ps", tag="ps")
                    for kc in range(KC):
                        nc.tensor.matmul(
                            ps[:],
                            wT_sb[kc][:, dc * P:(dc + 1) * P],
                            xTn[kc][:],
                            start=(kc == 0),
                            stop=(kc == KC - 1),
                        )
                    if dc % 2 == 0:
                        nc.scalar.copy(qkvT_sb[dc][:, nch * 512:(nch + 1) * 512], ps[:])
                    else:
                        nc.vector.tensor_copy(qkvT_sb[dc][:, nch * 512:(nch + 1) * 512], ps[:])
                # |sigma| broadcast chunk: row 0 of qkvT -> all 128 partitions
                sg = phA.tile([1, 512], BF16, name="sg", tag="sg", bufs=2)
                nc.scalar.activation(
                    sg[:], qkvT_sb[0][0:1, nch * 512:(nch + 1) * 512],
                    mybir.ActivationFunctionType.Abs,
                )
                pb = psum.tile([P, 512], F32, name="ps", tag="ps")
                nc.tensor.matmul(pb[:], ones[:], sg[:], start=True, stop=True)
                nc.vector.tensor_copy(sigbc[:, nch * 512:(nch + 1) * 512], pb[:])
                # logits for the 4 j-chunks covered by this n-chunk
                for jl in range(4):
                    jc = nch * 4 + jl
                    ps = psum.tile([P, BLK], F32, name="ps", tag="ps")
                    for dc in range(KC):
                        nc.tensor.matmul(
                            ps[:],
                            qkvT_sb[dc][:, jc * P:(jc + 1) * P],
                            rhs_i[dc][:],
                            start=(dc == 0), stop=(dc == KC - 1),
                        )
                    et = sb.tile([P, BLK], BF16, name=f"expT{jc}", tag=f"expT{jc}")
                    nc.scalar.activation(
                        et[:], ps[:], mybir.ActivationFunctionType.Exp,
                        scale=INV_SQRT_D, accum_out=pd[:, jc:jc + 1],
                    )
                    expT.append(et)


        # ---- P rows (emitted before the AllReduce: hides its latency)
        if True:
            for ic in range(IC):
                rs = sb.tile([P, NH], F32, name=f"rs{ic}", tag=f"rs{ic}")
                gaus = []
                for h in range(NH):
                    j0 = h * HW_
                    nz = pstream.tile([P, HW_], F32, name="nz", tag="nz")
                    nc.sync.dma_start(
                        nz[:], noise[ic * P:(ic + 1) * P, j0:j0 + HW_]
                    )
                    pabs = pstream.tile([P, HW_], F32, name="pabs", tag="pabs")
                    nc.gpsimd.iota(
                        pabs[:], pattern=[[-1, HW_]], base=ic * P - j0,
                        channel_multiplier=1, allow_small_or_imprecise_dtypes=True,
                    )
                    nc.scalar.activation(
                        pabs[:], pabs[:], mybir.ActivationFunctionType.Abs,
                        bias=ioff_sb[:, 0:1],
                    )
                    gau = pstream.tile([P, HW_], F32, name="gau", tag="gau", bufs=NH + 1)
                    nc.gpsimd.tensor_tensor(
                        gau[:], nz[:], sigbc[:, j0:j0 + HW_], mybir.AluOpType.mult
                    )
                    nc.vector.scalar_tensor_tensor(
                        gau[:], gau[:], 0.0, pabs[:],
                        op0=mybir.AluOpType.add, op1=mybir.AluOpType.add,
                        accum_out=rs[:, h:h + 1],
                    )
                    gaus.append(gau)
                rsum = sb.tile([P, 1], F32, name=f"rsum{ic}", tag=f"rsum{ic}")
                nc.vector.tensor_reduce(
                    rsum[:], rs[:], axis=mybir.AxisListType.X, op=mybir.AluOpType.add
                )
                rr = sb.tile([P, 1], F32, name=f"rr{ic}", tag=f"rr{ic}")
                nc.vector.reciprocal(rr[:], rsum[:])
                for h in range(NH):
                    j0 = h * HW_
                    nc.vector.tensor_scalar_mul(gaus[h][:], gaus[h][:], rr[:, 0:1])
                    nc.sync.dma_start(
                        out_p[ic * P:(ic + 1) * P, j0:j0 + HW_], gaus[h][:]
                    )

        # ---- AllReduce the softmax denominators ---------------------
        cc_in = dram.tile([P, JC], F32, name="cc_in")
        cc_out = dram.tile([P, JC], F32, name="cc_out", addr_space="Shared")
        nc.sync.dma_start(cc_in[:], pd[:])
        nc.gpsimd.collective_compute(
            "AllReduce",
            mybir.AluOpType.add,
            replica_groups=[list(range(NC))],
            ins=[cc_in[:]],
            outs=[cc_out[:]],
        )
        pd_full = sb.tile([P, JC], F32, name="pd_full", tag="pd_full")
        nc.sync.dma_start(pd_full[:], cc_out[:])
        rd = sb.tile([P, JC], F32, name="rd", tag="rd")
        nc.vector.reciprocal(rd[:], pd_full[:])

        # ---- normalize S.T in place ---------------------------------
        for jc in range(JC):
            nc.vector.tensor_scalar_mul(expT[jc][:], expT[jc][:], rd[:, jc:jc + 1])

        # ---- Z.T block: 4 PSUM banks accumulate in parallel ---------
        psz = [
            psum.tile([P, BLK], F32, name=f"psz{dc}", tag=f"psz{dc}", bufs=1)
            for dc in range(KC)
        ]
        for jc in range(JC):
            for dc in range(KC):
                nc.tensor.matmul(
                    psz[dc][:], qn_sb[jc][:, dc * P:(dc + 1) * P], expT[jc][:],
                    start=(jc == 0), stop=(jc == JC - 1),
                )
        with tc.tile_pool(name="zout", bufs=2) as zout:
            for dc in range(KC):
                zt = zout.tile([P, BLK], F32, name="zt_cp", tag="zt_cp")
                nc.scalar.copy(zt[:], psz[dc][:])
                nc.sync.dma_start(out_zt[dc * P:(dc + 1) * P, :], zt[:])

    nc.compile()
    return nc


def _get_compiled():
    global _compiled
    if _compiled is None:
        _compiled = _build()
    return _compiled


def _make_noise():
    import jax
    import jax.numpy as jnp

    return np.asarray(
        jax.random.normal(jax.random.key(42), (N, N), dtype=jnp.float32)
    )


def make_in_maps(x, W, noise):
    bf = ml_dtypes.bfloat16
    xT = np.ascontiguousarray(x.T.astype(bf))
    wT = np.ascontiguousarray(W.T.astype(bf))
    in_maps = []
    for c in range(NC):
        in_maps.append({
            "xT": xT,
            "xTs": np.ascontiguousarray(xT[:, c * BLK:(c + 1) * BLK]),
            "wT": wT,
            "noise": np.ascontiguousarray(noise[c * BLK:(c + 1) * BLK, :]),
            "ioff": np.full((P, 1), c * BLK, dtype=np.float32),
        })
    return in_maps


def assemble(results):
    Z = np.concatenate([results[c]["zt"].T for c in range(NC)], axis=0)
    Pm = np.concatenate([results[c]["p"] for c in range(NC)], axis=0)
    return Z, Pm


def kernel(x, W):
    x = np.ascontiguousarray(np.asarray(x, dtype=np.float32))
    W = np.ascontiguousarray(np.asarray(W, dtype=np.float32))
    noise = _make_noise()
    nc = _get_compiled()
    in_maps = make_in_maps(x, W, noise)
    res = bass_utils.run_bass_kernel_spmd(
        nc, in_maps, core_ids=list(range(NC)), trace=False
    )
    return assemble(res.results)


# revision 16
# speedup vs baseline: 2.3495x; 1.1167x over previous
"""AnomalyAttention distributed Bass kernel for 8 TRN2 NeuronCores.

Reference computation (n=4096, d=512):
    qkv = x @ W.T                       # [n, d];  Q = K = V = sigma = qkv
    L   = (Q @ K.T) / sqrt(d)           # [n, n]
    S   = softmax(L, axis=0)            # column softmax
    Z   = S @ V                         # [n, d]
    p[i,j]    = |i - j|
    gaussian  = p + |sigma[:,0]|[None,:] * noise      # noise = fixed jax key(42)
    P   = gaussian / gaussian.sum(-1, keepdims=True)  # row normalized
    returns (Z, P)

Sharding: each core owns a 512-row block i_block = [c*512, (c+1)*512).
Logits are built transposed, L.T[j, i_local] (all j on partitions, local i on
free), so the column softmax reduces along the free axis per partition; the
cross-core part of the reduction is a single 16 KiB AllReduce of per-column
partial sums, hidden behind the P-path (prior matrix) work.  Both layouts of
qkv (transposed for the logits lhsT, natural for Z.T = qkv.T @ S.T) are
computed on-chip in bf16 and stay SBUF-resident.  |i-j| comes from an
on-device iota + scalar-engine Abs with per-partition bias; gaussian and its
row-sum are fused DVE ops; the 1/rowsum scale runs on the scalar engine.
"""

import sys

if "/opt/trn_rl_repo" not in sys.path:
    sys.path.insert(0, "/opt/trn_rl_repo")

from contextlib import ExitStack

import ml_dtypes
import numpy as np

import concourse.bass as bass
import concourse.tile as tile
from concourse import bacc, mybir, bass_utils

N = 4096
D = 512
NC = 8
BLK = N // NC          # 512 rows of S / P per core
P = 128                # partitions
F32 = mybir.dt.float32
BF16 = mybir.dt.bfloat16
INV_SQRT_D = 1.0 / np.sqrt(D)

KC = D // P            # 4 contraction chunks of 128
JC = N // P            # 32 j-chunks of 128
NCH = N // 512         # 8 n-chunks of 512
IC = BLK // P          # 4 local i-chunks of 128
HW_ = 1024             # P-path free-dim tile
NH = N // HW_          # free chunks per i-chunk row

_compiled = None


def _build():
    nc = bacc.Bacc("TRN2", target_bir_lowering=False, debug=False, num_devices=NC)

    # Per-core inputs (bf16 except the P-path data).  xT/wT hold the same
    # data on every core; xTs/noise/ioff are per-core shards.
    xT = nc.dram_tensor("xT", [D, N], BF16, kind="ExternalInput").ap()      # x.T
    xTs = nc.dram_tensor("xTs", [D, BLK], BF16, kind="ExternalInput").ap()  # x.T[:, i_block]
    wT = nc.dram_tensor("wT", [D, D], BF16, kind="ExternalInput").ap()      # W.T
    noise = nc.dram_tensor("noise", [BLK, N], F32, kind="ExternalInput").ap()
    ioff = nc.dram_tensor("ioff", [P, 1], F32, kind="ExternalInput").ap()   # c*BLK

    out_zt = nc.dram_tensor("zt", [D, BLK], F32, kind="ExternalOutput").ap()  # Z.T block
    out_p = nc.dram_tensor("p", [BLK, N], F32, kind="ExternalOutput").ap()    # P rows

    with tile.TileContext(nc) as tc, ExitStack() as big:
        sb = big.enter_context(tc.tile_pool(name="sb", bufs=1))
        psum = big.enter_context(tc.tile_pool(name="psum", bufs=4, space="PSUM"))
        dram = big.enter_context(tc.tile_pool(name="dram", bufs=1, space="DRAM"))
        pstream = big.enter_context(tc.tile_pool(name="pstream", bufs=2))

        # bf16 SBUF residents: qkvT (4 x [128, N]) and natural qkv (32 x [128, D])
        qkvT_sb = [
            sb.tile([P, N], BF16, name=f"qkvT{dc}", tag=f"qkvT{dc}") for dc in range(KC)
        ]
        qn_sb = [
            sb.tile([P, D], BF16, name=f"qn{jc}", tag=f"qn{jc}") for jc in range(JC)
        ]
        sigbc = sb.tile([P, N], BF16, name="sigbc", tag="sigbc")
        ones = sb.tile([1, P], BF16, name="ones", tag="ones")
        nc.vector.memset(ones[:], 1.0)
        ioff_sb = sb.tile([P, 1], F32, name="ioff_sb", tag="ioff_sb")
        nc.sync.dma_start(ioff_sb[:], ioff[:])
        pd = sb.tile([P, JC], F32, name="pd", tag="pd")
        rhs_i = []
        expT = []

        with tc.tile_pool(name="phA", bufs=1) as phA:
            # ---- load W.T, x.T slice ------------------------------------
            wT_sb, xTs_sb = [], []
            for kc in range(KC):
                w = phA.tile([P, D], BF16, name=f"wT{kc}", tag=f"wT{kc}")
                nc.sync.dma_start(w[:], wT[kc * P:(kc + 1) * P, :])
                wT_sb.append(w)
                s = phA.tile([P, BLK], BF16, name=f"xTs{kc}", tag=f"xTs{kc}")
                nc.sync.dma_start(s[:], xTs[kc * P:(kc + 1) * P, :])
                xTs_sb.append(s)

            # ---- rhs_i = qkvT[:, i_block] (bf16, SBUF) ------------------
            for dc in range(KC):
                ps = psum.tile([P, BLK], F32, name="ps", tag="ps")
                for kc in range(KC):
                    nc.tensor.matmul(
                        ps[:],
                        wT_sb[kc][:, dc * P:(dc + 1) * P],
                        xTs_sb[kc][:],
                        start=(kc == 0),
                        stop=(kc == KC - 1),
                    )
                rt = sb.tile([P, BLK], BF16, name=f"rhs_i{dc}", tag=f"rhs_i{dc}")
                nc.scalar.copy(rt[:], ps[:])
                rhs_i.append(rt)

            # ---- natural qkv shard -> AllGather -> qn_sb -----------------
            # qkv[i_block, :] = x[i_block, :] @ W.T, gathered across cores.
            ag_in = dram.tile([BLK, D], BF16, name="ag_in")
            ag_out2 = dram.tile([N, D], BF16, name="ag_out", addr_space="Shared")
            for jl in range(4):
                ps = psum.tile([P, D], F32, name="ps", tag="ps")
                for kc in range(KC):
                    nc.tensor.matmul(
                        ps[:],
                        xTs_sb[kc][:, jl * P:(jl + 1) * P],
                        wT_sb[kc][:],
                        start=(kc == 0),
                        stop=(kc == KC - 1),
                    )
                qs = phA.tile([P, D], BF16, name="qshard", tag="qshard", bufs=2)
                nc.scalar.copy(qs[:], ps[:])
                nc.sync.dma_start(ag_in[jl * P:(jl + 1) * P, :], qs[:])
            nc.gpsimd.collective_compute(
                "AllGather",
                mybir.AluOpType.bypass,
                replica_groups=[list(range(NC))],
                ins=[ag_in[:]],
                outs=[ag_out2[:]],
            )

            # ---- qkvT full -> SBUF bf16, interleaved with logits + exp +
            # ---- partial column sums + per-chunk |sigma| broadcast ------
            nz_tiles = {}
            for nch in range(NCH):
                # pace two noise-row loads per n-chunk so the P-path streams
                k0 = nch * 2
                for k in (k0, k0 + 1):
                    ic, h = divmod(k, NH)
                    nz = pstream.tile([P, HW_], F32, name="nz", tag="nz", bufs=6)
                    nc.sync.dma_start(
                        nz[:], noise[ic * P:(ic + 1) * P, h * HW_:(h + 1) * HW_]
                    )
                    nz_tiles[(ic, h)] = nz
                xTn = []
                for kc in range(KC):
                    t = phA.tile([P, 512], BF16, name="xTn", tag=f"xTn{kc}", bufs=3)
                    nc.sync.dma_start(
                        t[:], xT[kc * P:(kc + 1) * P, nch * 512:(nch + 1) * 512]
                    )
                    xTn.append(t)
                for dc in range(KC):
                    ps = psum.tile([P, 512], F32, name="ps", tag="ps")
                    for kc in range(KC):
                        nc.tensor.matmul(
                            ps[:],
                            wT_sb[kc][:, dc * P:(dc + 1) * P],
                            xTn[kc][:],
                            start=(kc == 0),
                            stop=(kc == KC - 1),
                        )
                    if dc % 2 == 0:
                        nc.scalar.copy(qkvT_sb[dc][:, nch * 512:(nch + 1) * 512], ps[:])
                    else:
                        nc.vector.tensor_copy(qkvT_sb[dc][:, nch * 512:(nch + 1) * 512], ps[:])
                # |sigma| broadcast chunk: row 0 of qkvT -> all 128 partitions
                sg = phA.tile([1, 512], BF16, name="sg", tag="sg", bufs=2)
                nc.scalar.activation(
                    sg[:], qkvT_sb[0][0:1, nch * 512:(nch + 1) * 512],
                    mybir.ActivationFunctionType.Abs,
                )
                pb = psum.tile([P, 512], F32, name="ps", tag="ps")
                nc.tensor.matmul(pb[:], ones[:], sg[:], start=True, stop=True)
                nc.vector.tensor_copy(sigbc[:, nch * 512:(nch + 1) * 512], pb[:])
                # logits for the 4 j-chunks covered by this n-chunk
                for jl in range(4):
                    jc = nch * 4 + jl
                    ps = psum.tile([P, BLK], F32, name="ps", tag="ps")
                    for dc in range(KC):
                        nc.tensor.matmul(
                            ps[:],
                            qkvT_sb[dc][:, jc * P:(jc + 1) * P],
                            rhs_i[dc][:],
                            start=(dc == 0), stop=(dc == KC - 1),
                        )
                    et = sb.tile([P, BLK], BF16, name=f"expT{jc}", tag=f"expT{jc}")
                    nc.scalar.activation(
                        et[:], ps[:], mybir.ActivationFunctionType.Exp,
                        scale=INV_SQRT_D, accum_out=pd[:, jc:jc + 1],
                    )
                    expT.append(et)


        # ---- qn readback before the P-output stream hogs the queue --
        for jc in range(JC):
            nc.sync.dma_start(qn_sb[jc][:], ag_out2[jc * P:(jc + 1) * P, :])

        # ---- P rows (emitted before the AllReduce: hides its latency)
        if True:
            for ic in range(IC):
                rs = sb.tile([P, NH], F32, name=f"rs{ic}", tag=f"rs{ic}")
                gaus = []
                for h in range(NH):
                    j0 = h * HW_
                    nz = nz_tiles[(ic, h)]
                    pabs = pstream.tile([P, HW_], F32, name="pabs", tag="pabs")
                    nc.gpsimd.iota(
                        pabs[:], pattern=[[-1, HW_]], base=ic * P - j0,
                        channel_multiplier=1, allow_small_or_imprecise_dtypes=True,
                    )
                    nc.scalar.activation(
                        pabs[:], pabs[:], mybir.ActivationFunctionType.Abs,
                        bias=ioff_sb[:, 0:1],
                    )
                    gau = pstream.tile([P, HW_], F32, name="gau", tag="gau", bufs=NH + 1)
                    nc.gpsimd.tensor_tensor(
                        gau[:], nz[:], sigbc[:, j0:j0 + HW_], mybir.AluOpType.mult
                    )
                    nc.vector.scalar_tensor_tensor(
                        gau[:], gau[:], 0.0, pabs[:],
                        op0=mybir.AluOpType.add, op1=mybir.AluOpType.add,
                        accum_out=rs[:, h:h + 1],
                    )
                    gaus.append(gau)
                rsum = sb.tile([P, 1], F32, name=f"rsum{ic}", tag=f"rsum{ic}")
                nc.vector.tensor_reduce(
                    rsum[:], rs[:], axis=mybir.AxisListType.X, op=mybir.AluOpType.add
                )
                rr = sb.tile([P, 1], F32, name=f"rr{ic}", tag=f"rr{ic}")
                nc.vector.reciprocal(rr[:], rsum[:])
                for h in range(NH):
                    j0 = h * HW_
                    nc.scalar.activation(
                        gaus[h][:], gaus[h][:], mybir.ActivationFunctionType.Copy,
                        scale=rr[:, 0:1],
                    )
                    nc.sync.dma_start(
                        out_p[ic * P:(ic + 1) * P, j0:j0 + HW_], gaus[h][:]
                    )

        # ---- AllReduce the softmax denominators ---------------------
        cc_in = dram.tile([P, JC], F32, name="cc_in")
        cc_out = dram.tile([P, JC], F32, name="cc_out", addr_space="Shared")
        nc.sync.dma_start(cc_in[:], pd[:])
        nc.gpsimd.collective_compute(
            "AllReduce",
            mybir.AluOpType.add,
            replica_groups=[list(range(NC))],
            ins=[cc_in[:]],
            outs=[cc_out[:]],
        )
        pd_full = sb.tile([P, JC], F32, name="pd_full", tag="pd_full")
        nc.sync.dma_start(pd_full[:], cc_out[:])
        rd = sb.tile([P, JC], F32, name="rd", tag="rd")
        nc.vector.reciprocal(rd[:], pd_full[:])

        # ---- normalize S.T in place ---------------------------------
        for jc in range(JC):
            nc.vector.tensor_scalar_mul(expT[jc][:], expT[jc][:], rd[:, jc:jc + 1])

        # ---- Z.T block: 4 PSUM banks accumulate in parallel ---------
        psz = [
            psum.tile([P, BLK], F32, name=f"psz{dc}", tag=f"psz{dc}", bufs=1)
            for dc in range(KC)
        ]
        for jc in range(JC):
            for dc in range(KC):
                nc.tensor.matmul(
                    psz[dc][:], qn_sb[jc][:, dc * P:(dc + 1) * P], expT[jc][:],
                    start=(jc == 0), stop=(jc == JC - 1),
                )
        with tc.tile_pool(name="zout", bufs=2) as zout:
            for dc in range(KC):
                zt = zout.tile([P, BLK], F32, name="zt_cp", tag="zt_cp")
                nc.scalar.copy(zt[:], psz[dc][:])
                nc.sync.dma_start(out_zt[dc * P:(dc + 1) * P, :], zt[:])

    nc.compile()
    return nc


def _get_compiled():
    global _compiled
    if _compiled is None:
        _compiled = _build()
    return _compiled


def _make_noise():
    import jax
    import jax.numpy as jnp

    return np.asarray(
        jax.random.normal(jax.random.key(42), (N, N), dtype=jnp.float32)
    )


def make_in_maps(x, W, noise):
    bf = ml_dtypes.bfloat16
    xT = np.ascontiguousarray(x.T.astype(bf))
    wT = np.ascontiguousarray(W.T.astype(bf))
    in_maps = []
    for c in range(NC):
        in_maps.append({
            "xT": xT,
            "xTs": np.ascontiguousarray(xT[:, c * BLK:(c + 1) * BLK]),
            "wT": wT,
            "noise": np.ascontiguousarray(noise[c * BLK:(c + 1) * BLK, :]),
            "ioff": np.full((P, 1), c * BLK, dtype=np.float32),
        })
    return in_maps


def assemble(results):
    Z = np.concatenate([results[c]["zt"].T for c in range(NC)], axis=0)
    Pm = np.concatenate([results[c]["p"] for c in range(NC)], axis=0)
    return Z, Pm


def kernel(x, W):
    x = np.ascontiguousarray(np.asarray(x, dtype=np.float32))
    W = np.ascontiguousarray(np.asarray(W, dtype=np.float32))
    noise = _make_noise()
    nc = _get_compiled()
    in_maps = make_in_maps(x, W, noise)
    res = bass_utils.run_bass_kernel_spmd(
        nc, in_maps, core_ids=list(range(NC)), trace=False
    )
    return assemble(res.results)
